# revision 29
# baseline (speedup 1.0000x reference)
"""Distributed Trainium2 Bass kernel for a post-LN transformer layer.

Reference computation (problem nn_AttentionLayer_257698038341):
    x: (L=2048, B=4, D=1024), H=16 heads, DFF=4096, fp32, exact GELU.
    q,k,v = x@W{q,k,v}+b ; attn = softmax(q k^T/sqrt(dk)) v ; post-LN; FFN; post-LN.

Sharding: sequence-parallel over L across 8 cores (256 rows of L each ->
1024 tokens per core, tokens ordered b-major). Each core holds the full
weights. The only collectives are two AllGathers (K^T and V-with-ones-column,
~2MB bf16 per rank each); attention, projections, LN and FFN are all local.

v2 (this file) vs the original baseline:
  * all PE operands are bf16 (weights cast on host; activations cast on the
    producing ACT/DVE op). fp32r matmuls measured ~1.9-3.8 cyc/row on HW;
    bf16 runs at 1 cyc/row and enables fast weight load.
  * weights are DMA'd once, contiguously (host pre-arranges the exact SBUF
    layout); no per-tile weight reloads.
  * the gathered K^T/V live in DRAM layouts chosen so phase B loads them with
    one large DMA per batch element (4KB/2KB descriptor runs), instead of
    9 small gather-DMAs per (b,h) with 256-512B runs.
  * softmax exp is batched 6/6/4 key-tiles per ACTIVATE ([128,1536] from a
    3-bank PSUM tile), 3 ACT instructions per head on the phase-B critical path.
  * attention heads are software-pipelined (S-matmuls of head h issue before
    the PV/normalize of head h-1) so the PE never waits on the exp.
  * projection biases fused into the ACT copy where per-partition.
"""

import sys
import os

for _p in ("/opt/trn_rl_repo",):
    if _p not in sys.path and os.path.isdir(_p):
        sys.path.insert(0, _p)

import numpy as np
from contextlib import ExitStack

from concourse import bacc, bass, tile, mybir, masks
from concourse.bass_utils import run_bass_kernel_spmd

F32 = mybir.dt.float32
F32R = mybir.dt.float32r
BF = mybir.dt.bfloat16
AF = mybir.ActivationFunctionType
OP = mybir.AluOpType

NCORES = 8
L, B, D, H = 2048, 4, 1024, 16
DK = D // H            # 64
DFF = 4 * D            # 4096
LS = L // NCORES       # 256 l-rows per core
NT = LS * B            # 1024 tokens per core
P = 128                # partitions
DCH = D // P           # 8 d-chunks
MFF = DFF // P         # 32 dff-chunks
EPS = 1e-5
VW = DK + 1            # 65: v columns per head incl. ones column
N2 = 512               # moving free dim for big matmuls
NKT = L // P           # 16 key tiles of 128
SCALE = 1.0 / np.sqrt(DK)

RG = [list(range(NCORES))]


def r32(ap):
    return ap.bitcast(F32R)


def build_nc():
    nc = bacc.Bacc("TRN2")

    xn_e = nc.declare_dram_parameter("xn", [NT, D], F32, isOutput=False)
    xt_e = nc.declare_dram_parameter("xt", [P, DCH * NT], BF, isOutput=False)
    wq_e = nc.declare_dram_parameter("wq", [P, DCH * DCH * P], BF, isOutput=False)
    wk_e = nc.declare_dram_parameter("wk", [P, DCH * DCH * P], BF, isOutput=False)
    wv_e = nc.declare_dram_parameter("wv", [P, DCH * D], BF, isOutput=False)
    wo_e = nc.declare_dram_parameter("wo", [P, DCH * D], BF, isOutput=False)
    w1_e = nc.declare_dram_parameter("w1", [P, MFF * DCH * P], BF, isOutput=False)
    w2_e = nc.declare_dram_parameter("w2", [P, 2 * MFF * N2], BF, isOutput=False)
    bq_e = nc.declare_dram_parameter("bq", [1, D], F32, isOutput=False)
    bk_e = nc.declare_dram_parameter("bk", [1, D], F32, isOutput=False)
    bv_e = nc.declare_dram_parameter("bv", [1, D], F32R, isOutput=False)
    bo_e = nc.declare_dram_parameter("bo", [1, D], F32R, isOutput=False)
    b1_e = nc.declare_dram_parameter("b1", [1, DFF], F32, isOutput=False)
    b2_e = nc.declare_dram_parameter("b2", [1, D], F32R, isOutput=False)
    g1_e = nc.declare_dram_parameter("g1", [1, D], F32R, isOutput=False)
    be1_e = nc.declare_dram_parameter("be1", [1, D], F32R, isOutput=False)
    g2_e = nc.declare_dram_parameter("g2", [1, D], F32R, isOutput=False)
    be2_e = nc.declare_dram_parameter("be2", [1, D], F32R, isOutput=False)
    ones_r_e = nc.declare_dram_parameter("ones_r", [1, P], F32R, isOutput=False)
    ones_c_e = nc.declare_dram_parameter("ones_c", [P, H], BF, isOutput=False)
    out_e = nc.declare_dram_parameter("out", [NT, D], F32, isOutput=True)

    with tile.TileContext(nc) as tc, ExitStack() as ctx:
        persist = ctx.enter_context(tc.tile_pool(name="persist", bufs=1))
        dram = ctx.enter_context(tc.tile_pool(name="dram", bufs=1, space="DRAM"))

        # gathered K^T: row (b*128 + h%2*64 + dk), col (h//2 * 256 + l256)
        kT_b = dram.tile([B * P, DCH * LS], BF)
        # gathered V': row (b*256 + c*128 + p), col (h*65 + e); e=64 is ones
        vp_b = dram.tile([NT, H * VW], BF)
        kT_full = dram.tile([NCORES * B * P, DCH * LS], BF, addr_space="Shared")
        vp_full = dram.tile([NCORES * NT, H * VW], BF, addr_space="Shared")

        # ---- constants ----
        ident = persist.tile([P, P], F32)
        masks.make_identity(nc, ident[:])
        ones_row = persist.tile([1, P], F32R)
        nc.sync.dma_start(ones_row[:], ones_r_e[0:1, :])
        ones_col = persist.tile([P, H], BF)
        nc.sync.dma_start(ones_col[:], ones_c_e[:])
        eps_t = persist.tile([P, 1], F32)
        nc.vector.memset(eps_t[:], EPS)

        # per-partition bias tiles: col m holds bias[m*128 : (m+1)*128]
        bq_pp = persist.tile([P, DCH], F32)
        nc.sync.dma_start(bq_pp[:], bq_e.rearrange("o (m p) -> (o p) m", p=P))
        bk_pp = persist.tile([P, DCH], F32)
        nc.sync.dma_start(bk_pp[:], bk_e.rearrange("o (m p) -> (o p) m", p=P))
        b1_pp = persist.tile([P, MFF], F32)
        nc.sync.dma_start(b1_pp[:], b1_e.rearrange("o (m p) -> (o p) m", p=P))

        # broadcast a [1, n] dram row across all 128 partitions (PE ones-matmul)
        def bcast_row(src_e, n, name, pool, row_pool, psum_pool):
            row = row_pool.tile([1, n], F32R, name=f"{name}_row", tag="row")
            nc.sync.dma_start(row[:], src_e[0:1, :])
            bc = pool.tile([P, n], F32, name=f"{name}_bc")
            for j in range(n // N2):
                ps = psum_pool.tile([P, N2], F32, name=f"{name}_ps{j}", tag="bc_ps")
                nc.tensor.matmul(ps[:], ones_row[:1, :],
                                 row[:1, j * N2:(j + 1) * N2],
                                 start=True, stop=True)
                nc.scalar.copy(bc[:, j * N2:(j + 1) * N2], ps[:])
            return bc

        # =============== phase A: QKV projections + AllGathers ===============
        stAB = ExitStack()
        poolAB = stAB.enter_context(tc.tile_pool(name="poolAB", bufs=1))
        qT = poolAB.tile([P, DCH, NT], BF)    # q^T: [h2*64+dk, hh, token]

        stA = ExitStack()
        poolA = stA.enter_context(tc.tile_pool(name="poolA", bufs=1))
        rowA = stA.enter_context(tc.tile_pool(name="rowA", bufs=2))
        wst = stA.enter_context(tc.tile_pool(name="wst", bufs=2))
        vpp = stA.enter_context(tc.tile_pool(name="vpp", bufs=3))
        psA = stA.enter_context(tc.tile_pool(name="psA", bufs=3, space="PSUM"))
        if True:
            stPA = ExitStack()
            bcPsA = stPA.enter_context(
                tc.tile_pool(name="bcPsA", bufs=2, space="PSUM"))
            bv_bc = bcast_row(bv_e, D, "bv", poolA, rowA, bcPsA)
            stPA.close()
            xT = poolA.tile([P, DCH, NT], BF)
            nc.sync.dma_start(xT[:], xt_e.rearrange("p (c n) -> p c n", n=NT))

            wk_sb = wst.tile([P, DCH, DCH, P], BF, name="wk_sb", tag="wstat")
            nc.sync.dma_start(wk_sb[:],
                              wk_e.rearrange("p (m k q) -> p m k q", k=DCH, q=P))
            wq_sb = wst.tile([P, DCH, DCH, P], BF, name="wq_sb", tag="wstat")
            nc.sync.dma_start(wq_sb[:],
                              wq_e.rearrange("p (m k q) -> p m k q", k=DCH, q=P))

            # --- K^T projection -> kT_sb -> kT_b -> AllGather(K) ---
            kT_sb = poolA.tile([P, DCH, NT], BF)
            for m in range(DCH):
                for half in range(2):
                    ps = psA.tile([P, N2], F32, name=f"ps_k{m}{half}", tag="psA")
                    for k in range(DCH):
                        nc.tensor.matmul(
                            ps[:], wk_sb[:, m, k, :],
                            xT[:, k, half * N2:(half + 1) * N2],
                            start=(k == 0), stop=(k == DCH - 1))
                    nc.scalar.activation(
                        kT_sb[:, m, half * N2:(half + 1) * N2], ps[:],
                        AF.Identity, bias=bk_pp[:, m:m + 1])
            kb_v = kT_b.rearrange("(b p) (hh l) -> b p hh l", p=P, l=LS)
            for b in range(B):
                nc.sync.dma_start(kb_v[b], kT_sb[:, :, b * LS:(b + 1) * LS])
            nc.gpsimd.collective_compute(
                "AllGather", OP.bypass, replica_groups=RG,
                ins=[kT_b.opt()], outs=[kT_full.opt()])

            # --- V projection (natural) -> vp_b -> AllGather(V) ---
            wv_sb = poolA.tile([P, DCH, D], BF)
            nc.sync.dma_start(wv_sb[:], wv_e.rearrange("p (k n) -> p k n", n=D))
            NQ = 256  # quarter of dout (4 heads)
            for tt in range(DCH):  # token tiles
                vp_t = vpp.tile([P, H * VW], BF, name="vp_t", tag="vp_t")
                nc.scalar.copy(
                    vp_t[:].rearrange("p (hh e) -> p hh e", e=VW)
                    [:, :, DK:DK + 1], ones_col[:, :, None])
                for qu in range(4):
                    ps = psA.tile([P, NQ], F32, name=f"ps_v{tt}{qu}", tag="psV")
                    for k in range(DCH):
                        nc.tensor.matmul(
                            ps[:], xT[:, k, tt * P:(tt + 1) * P],
                            wv_sb[:, k, qu * NQ:(qu + 1) * NQ],
                            start=(k == 0), stop=(k == DCH - 1))
                    dst = vp_t[:, qu * 4 * VW:(qu + 1) * 4 * VW] \
                        .rearrange("p (hh e) -> p hh e", e=VW)[:, :, 0:DK]
                    nc.vector.tensor_tensor(
                        dst,
                        ps[:].rearrange("p (hh e) -> p hh e", e=DK),
                        bv_bc[:, qu * NQ:(qu + 1) * NQ]
                        .rearrange("p (hh e) -> p hh e", e=DK),
                        op=OP.add)
                nc.sync.dma_start(vp_b[tt * P:(tt + 1) * P, :], vp_t[:])
            nc.gpsimd.collective_compute(
                "AllGather", OP.bypass, replica_groups=RG,
                ins=[vp_b.opt()], outs=[vp_full.opt()])

            # --- Q^T projection -> qT (SBUF resident) ---
            for m in range(DCH):
                for half in range(2):
                    ps = psA.tile([P, N2], F32, name=f"ps_q{m}{half}", tag="psA")
                    for k in range(DCH):
                        nc.tensor.matmul(
                            ps[:], wq_sb[:, m, k, :],
                            xT[:, k, half * N2:(half + 1) * N2],
                            start=(k == 0), stop=(k == DCH - 1))
                    nc.scalar.activation(
                        qT[:, m, half * N2:(half + 1) * N2], ps[:],
                        AF.Identity, bias=bq_pp[:, m:m + 1])

        stA.close()
        # tiles that must survive into phase D (opened before poolBC so the
        # pool stack stays LIFO: poolBC closes after C, poolCD after D)
        stCD = ExitStack()
        poolCD = stCD.enter_context(tc.tile_pool(name="poolCD", bufs=1))
        y1 = poolCD.tile([P, DCH, D], F32)       # LN1 out natural [tok, tt, d]
        y1T = poolCD.tile([P, DCH, NT], BF)      # LN1 out transposed [d, k, tok]

        # ======================= phase B: attention =======================
        stBC = ExitStack()
        poolBC = stBC.enter_context(tc.tile_pool(name="poolBC", bufs=1))
        attnT = poolBC.tile([P, DCH, NT], BF)   # [h2*64+dk, hh, token]
        kv_view = kT_full.rearrange("(r b p) n -> b p r n", b=B, p=P)
        vp_view = vp_full.rearrange("(r b c p) n -> b p r c n", b=B, c=2, p=P)
        stB = ExitStack()
        kvp = stB.enter_context(tc.tile_pool(name="kvp", bufs=3))
        pbp = stB.enter_context(tc.tile_pool(name="pbp", bufs=2))
        nrm = stB.enter_context(tc.tile_pool(name="nrm", bufs=3))
        psS = stB.enter_context(tc.tile_pool(name="psS", bufs=2, space="PSUM"))
        psAcc = stB.enter_context(tc.tile_pool(name="psAcc", bufs=2, space="PSUM"))
        if True:
            # K/V for one batch element arrive as two half-tiles (ranks 0-3 /
            # 4-7) so three half-buffers give cross-b prefetch within SBUF.
            # software pipeline: S-matmuls + exp of head i, then PV+normalize
            # of head i-1 (keeps the PE fed while the exp runs on ACT).
            RH = NCORES // 2
            pend = None

            def pv_normalize(b, h, acc):
                hh, h2 = h // 2, h % 2
                b0 = h2 * DK
                rec = nrm.tile([1, LS], F32, name="rec", tag="rec")
                nc.vector.reciprocal(rec[:], acc[DK:DK + 1, :])
                bcr = nrm.tile([DK, LS], F32, name="bcr", tag="bcr")
                nc.gpsimd.partition_broadcast(bcr[:], rec[:])
                nc.vector.tensor_tensor(
                    attnT[b0:b0 + DK, hh, b * LS:(b + 1) * LS],
                    acc[0:DK, :], bcr[:], op=OP.mult)

            for b in range(B):
                k_h = []
                v_h = []
                for rh in range(2):
                    kt_t = kvp.tile([P, RH, DCH, LS], BF, name=f"k_sb{rh}",
                                    tag="k_sb")
                    nc.sync.dma_start(
                        kt_t[:].rearrange("p r hh l -> p r (hh l)"),
                        kv_view[b][:, rh * RH:(rh + 1) * RH])
                    k_h.append(kt_t)
                    vt_t = kvp.tile([P, RH, 2, H * VW], BF, name=f"v_sb{rh}",
                                    tag="v_sb")
                    for c in range(2):
                        nc.sync.dma_start(
                            vt_t[:, :, c, :],
                            vp_view[b][:, rh * RH:(rh + 1) * RH, c, :])
                    v_h.append(vt_t)
                for h in range(H):
                    hh, h2 = h // 2, h % 2
                    b0 = h2 * DK
                    q_sl = qT[b0:b0 + DK, hh, b * LS:(b + 1) * LS]
                    p_bh = pbp.tile([P, NKT, LS], BF, name="p_bh", tag="p_bh")
                    acc = psAcc.tile([VW, LS], F32, name="acc65", tag="acc65")
                    # 3 exp batches per head (6/6/4 key-tiles; [P,1536] = 3
                    # PSUM banks x 2 bufs + 2 acc banks = all 8) — fewer
                    # ACTIVATE instructions on the phase-B-critical ACT engine
                    kt0 = 0
                    for gn in (6, 6, 4):
                        ps = psS.tile([P, 6 * LS], F32, name="s_ps", tag="s_ps")
                        for j in range(gn):
                            kt = kt0 + j
                            r, c = divmod(kt, 2)
                            nc.tensor.matmul(
                                ps[:, j * LS:(j + 1) * LS],
                                k_h[r // RH][b0:b0 + DK, r % RH, hh,
                                             c * P:(c + 1) * P],
                                q_sl, start=True, stop=True)
                        nc.scalar.activation(
                            p_bh[:, kt0:kt0 + gn, :]
                            .rearrange("p g l -> p (g l)"),
                            ps[:, :gn * LS], AF.Exp, scale=float(SCALE))
                        kt0 += gn
                    if pend is not None:
                        pb_, acc_, vh_, b_, h_ = pend
                        for kt in range(NKT):
                            r, c = divmod(kt, 2)
                            nc.tensor.matmul(
                                acc_[:], vh_[r // RH][:, r % RH, c,
                                                      h_ * VW:(h_ + 1) * VW],
                                pb_[:, kt, :],
                                start=(kt == 0), stop=(kt == NKT - 1))
                        pv_normalize(b_, h_, acc_)
                    pend = (p_bh, acc, v_h, b, h)
            # epilogue: last head
            pb_, acc_, vh_, b_, h_ = pend
            for kt in range(NKT):
                r, c = divmod(kt, 2)
                nc.tensor.matmul(
                    acc_[:], vh_[r // RH][:, r % RH, c, h_ * VW:(h_ + 1) * VW],
                    pb_[:, kt, :], start=(kt == 0), stop=(kt == NKT - 1))
            pv_normalize(b_, h_, acc_)

        stB.close()
        # ================ phase C: O-projection + LN1 + transpose ================
        stC = ExitStack()
        poolC = stC.enter_context(tc.tile_pool(name="poolC", bufs=2))
        bcC = stC.enter_context(tc.tile_pool(name="bcC", bufs=1))
        rowC = stC.enter_context(tc.tile_pool(name="rowC", bufs=2))
        wC = stC.enter_context(tc.tile_pool(name="wC", bufs=1))
        if True:
            stPC = ExitStack()
            bcPsC = stPC.enter_context(
                tc.tile_pool(name="bcPsC", bufs=2, space="PSUM"))
            bo_bc = bcast_row(bo_e, D, "bo", bcC, rowC, bcPsC)
            g1_bc = bcast_row(g1_e, D, "g1", bcC, rowC, bcPsC)
            be1_bc = bcast_row(be1_e, D, "be1", bcC, rowC, bcPsC)
            stPC.close()
            psO = stC.enter_context(tc.tile_pool(name="psO", bufs=3, space="PSUM"))
            psT = stC.enter_context(tc.tile_pool(name="psT", bufs=4, space="PSUM"))
            wo_sb = wC.tile([P, DCH, D], BF)
            nc.sync.dma_start(wo_sb[:], wo_e.rearrange("p (k n) -> p k n", n=D))
            for tt in range(DCH):
                xn_t = poolC.tile([P, D], F32, name="xn_t", tag="xn_t")
                nc.sync.dma_start(xn_t[:], xn_e[tt * P:(tt + 1) * P, :])
                xb = poolC.tile([P, D], F32, name="xb", tag="xb")
                nc.vector.tensor_tensor(xb[:], xn_t[:], bo_bc[:], op=OP.add)
                res1 = poolC.tile([P, D], F32, name="res1", tag="res1")
                for half in range(2):
                    sl = slice(half * N2, (half + 1) * N2)
                    ps = psO.tile([P, N2], F32, name="o_ps", tag="o_ps")
                    for k in range(DCH):
                        nc.tensor.matmul(
                            ps[:], attnT[:, k, tt * P:(tt + 1) * P],
                            wo_sb[:, k, sl],
                            start=(k == 0), stop=(k == DCH - 1))
                    nc.vector.tensor_tensor(res1[:, sl], ps[:], xb[:, sl],
                                            op=OP.add)
                # LN1
                stats = poolC.tile([P, 2, 6], F32, name="stats1", tag="stats")
                for c_ in range(2):
                    nc.vector.bn_stats(stats[:, c_, :],
                                       res1[:, c_ * N2:(c_ + 1) * N2])
                aggr = poolC.tile([P, 2], F32, name="aggr1", tag="aggr")
                nc.vector.bn_aggr(aggr[:], stats[:])
                std = poolC.tile([P, 1], F32, name="std1", tag="std")
                nc.scalar.activation(std[:], aggr[:, 1:2], AF.Sqrt,
                                     bias=eps_t[:])
                rstd = poolC.tile([P, 1], F32, name="rstd1", tag="rstd")
                nc.vector.reciprocal(rstd[:], std[:])
                y1t_n = y1[:, tt, :]
                nc.vector.tensor_scalar(y1t_n, res1[:], aggr[:, 0:1], rstd[:],
                                        op0=OP.subtract, op1=OP.mult)
                nc.vector.tensor_tensor(y1t_n, y1t_n, g1_bc[:], op=OP.mult)
                nc.vector.tensor_tensor(y1t_n, y1t_n, be1_bc[:], op=OP.add)
                # transpose y1 tile -> y1T (PE, bf16)
                for k in range(DCH):
                    tps = psT.tile([P, P], F32, name="t_ps", tag="t_ps")
                    nc.tensor.transpose(tps[:], y1[:, tt, k * P:(k + 1) * P],
                                        ident[:])
                    nc.scalar.copy(y1T[:, k, tt * P:(tt + 1) * P], tps[:])

        stC.close()
        stBC.close()
        # ======================= phase D: FFN + LN2 =======================
        NTB = NT // N2  # 2 blocks of 512 tokens
        stD = ExitStack()
        hp = stD.enter_context(tc.tile_pool(name="hp", bufs=1))
        bcD = stD.enter_context(tc.tile_pool(name="bcD", bufs=1))
        rowD = stD.enter_context(tc.tile_pool(name="rowD", bufs=2))
        w1p = stD.enter_context(tc.tile_pool(name="w1p", bufs=2))
        w2p = stD.enter_context(tc.tile_pool(name="w2p", bufs=2))
        poolD = stD.enter_context(tc.tile_pool(name="poolD", bufs=2))
        if True:
            stPD = ExitStack()
            bcPsD = stPD.enter_context(
                tc.tile_pool(name="bcPsD", bufs=2, space="PSUM"))
            b2_bc = bcast_row(b2_e, D, "b2", bcD, rowD, bcPsD)
            g2_bc = bcast_row(g2_e, D, "g2", bcD, rowD, bcPsD)
            be2_bc = bcast_row(be2_e, D, "be2", bcD, rowD, bcPsD)
            stPD.close()
            psH = stD.enter_context(tc.tile_pool(name="psH", bufs=3, space="PSUM"))
            psF = stD.enter_context(tc.tile_pool(name="psF", bufs=1, space="PSUM"))
            h_sb = hp.tile([P, MFF, N2], BF)  # gelu out for one 512-token block
            w1_v = w1_e.rearrange("p (m k q) -> p m k q", k=DCH, q=P)
            w2_v = w2_e.rearrange("p (h c n) -> p h c n", h=2, n=N2)
            for tb in range(NTB):
                tsl = slice(tb * N2, (tb + 1) * N2)
                # FFN1: h^T[m, tsl], streaming w1 in 4 chunks of 8 m's
                for mc in range(4):
                    w1_blk = w1p.tile([P, 8, DCH, P], BF, name="w1_blk",
                                      tag="w1")
                    nc.sync.dma_start(w1_blk[:], w1_v[:, mc * 8:(mc + 1) * 8])
                    for mm in range(8):
                        m = mc * 8 + mm
                        ps = psH.tile([P, N2], F32, name="h_ps", tag="h_ps")
                        for k in range(DCH):
                            nc.tensor.matmul(ps[:], w1_blk[:, mm, k, :],
                                             y1T[:, k, tsl],
                                             start=(k == 0), stop=(k == DCH - 1))
                        nc.scalar.activation(h_sb[:, m, :], ps[:], AF.Gelu,
                                             bias=b1_pp[:, m:m + 1])
                # FFN2: per output half, accumulate over all 32 dff chunks
                # into 4 token-tile PSUM banks; w2 streamed in 8-chunk pieces
                res2s = [poolD.tile([P, D], F32, name=f"res2_{tb}{q}",
                                    tag=f"res2{q}", bufs=1) for q in range(4)]
                for half in range(2):
                    sl = slice(half * N2, (half + 1) * N2)
                    accs = [psF.tile([P, N2], F32, name=f"f_ps{q}",
                                     tag=f"f_ps{q}", bufs=1) for q in range(4)]
                    for cc in range(4):
                        w2_c = w2p.tile([P, 8, N2], BF, name="w2_c", tag="w2")
                        nc.sync.dma_start(w2_c[:],
                                          w2_v[:, half, cc * 8:(cc + 1) * 8])
                        for c8 in range(8):
                            c = cc * 8 + c8
                            for q in range(4):
                                nc.tensor.matmul(
                                    accs[q][:],
                                    h_sb[:, c, q * P:(q + 1) * P],
                                    w2_c[:, c8, :],
                                    start=(c == 0), stop=(c == MFF - 1))
                    for q in range(4):
                        tt = tb * 4 + q
                        nc.vector.tensor_tensor(res2s[q][:, sl], accs[q][:],
                                                b2_bc[:, sl], op=OP.add)
                        nc.vector.tensor_tensor(res2s[q][:, sl], res2s[q][:, sl],
                                                y1[:, tt, sl], op=OP.add)
                for q in range(4):
                    tt = tb * 4 + q
                    res2 = res2s[q]
                    stats = poolD.tile([P, 2, 6], F32, name="stats2",
                                       tag="stats2")
                    for c_ in range(2):
                        nc.vector.bn_stats(stats[:, c_, :],
                                           res2[:, c_ * N2:(c_ + 1) * N2])
                    aggr = poolD.tile([P, 2], F32, name="aggr2", tag="aggr2")
                    nc.vector.bn_aggr(aggr[:], stats[:])
                    std = poolD.tile([P, 1], F32, name="std2", tag="std2")
                    nc.scalar.activation(std[:], aggr[:, 1:2], AF.Sqrt,
                                         bias=eps_t[:])
                    rstd = poolD.tile([P, 1], F32, name="rstd2", tag="rstd2")
                    nc.vector.reciprocal(rstd[:], std[:])
                    o_t = poolD.tile([P, D], F32, name="o_t", tag="o_t")
                    nc.vector.tensor_scalar(o_t[:], res2[:], aggr[:, 0:1],
                                            rstd[:],
                                            op0=OP.subtract, op1=OP.mult)
                    nc.vector.tensor_tensor(o_t[:], o_t[:], g2_bc[:], op=OP.mult)
                    nc.vector.tensor_tensor(o_t[:], o_t[:], be2_bc[:], op=OP.add)
                    nc.sync.dma_start(out_e[tt * P:(tt + 1) * P, :], o_t[:])
        stD.close()
        stCD.close()
        stAB.close()

    nc.finalize()
    return nc


def make_in_maps(inputs):
    import ml_dtypes
    bf16 = ml_dtypes.bfloat16
    x = np.ascontiguousarray(np.asarray(inputs["x"], dtype=np.float32))
    w = {k: np.asarray(v, dtype=np.float32) for k, v in inputs.items()
         if k != "x"}

    def stat(wm, mout):  # [D, mout*P] -> [p][m, k, q] (stationary chunks)
        w4 = wm.reshape(DCH, P, mout, P)
        return np.ascontiguousarray(
            w4.transpose(1, 2, 0, 3).reshape(P, -1).astype(bf16))

    def mov(wm):  # [D, n] -> [p][k, n] (moving slices)
        w3 = wm.reshape(DCH, P, wm.shape[1])
        return np.ascontiguousarray(
            w3.transpose(1, 0, 2).reshape(P, -1).astype(bf16))

    w2f = w["W2"].reshape(MFF, P, 2, N2)     # [c, p, half, n]
    w2h = np.ascontiguousarray(
        w2f.transpose(1, 2, 0, 3).reshape(P, -1).astype(bf16))

    row = lambda a: np.ascontiguousarray(a.reshape(1, -1))
    shared = dict(
        wq=stat(w["Wq"], DCH), wk=stat(w["Wk"], DCH), wv=mov(w["Wv"]),
        wo=mov(w["Wo"]), w1=stat(w["W1"], MFF), w2=w2h,
        bq=row(w["bq"]), bk=row(w["bk"]), bv=row(w["bv"]), bo=row(w["bo"]),
        b1=row(w["b1"]), b2=row(w["b2"]), g1=row(w["g1"]), be1=row(w["be1"]),
        g2=row(w["g2"]), be2=row(w["be2"]),
        ones_r=np.ones((1, P), np.float32),
        ones_c=np.ones((P, H), bf16),
    )
    in_maps = []
    for i in range(NCORES):
        xn = np.ascontiguousarray(
            x[i * LS:(i + 1) * LS].transpose(1, 0, 2).reshape(NT, D))
        xt4 = xn.T.reshape(DCH, P, NT)
        xt = np.ascontiguousarray(
            xt4.transpose(1, 0, 2).reshape(P, -1).astype(bf16))
        m = dict(shared)
        m["xn"] = xn
        m["xt"] = xt
        in_maps.append(m)
    return in_maps


def assemble(results):
    full = np.empty((L, B, D), np.float32)
    for i in range(NCORES):
        shard = results[i]["out"]
        full[i * LS:(i + 1) * LS] = shard.reshape(B, LS, D).transpose(1, 0, 2)
    return full


_NC_CACHE = None


def _get_nc():
    global _NC_CACHE
    if _NC_CACHE is None:
        _NC_CACHE = build_nc()
    return _NC_CACHE


def kernel(**inputs):
    nc = _get_nc()
    in_maps = make_in_maps(inputs)
    res = run_bass_kernel_spmd(nc, in_maps, list(range(NCORES)))
    return assemble(res.results)


if __name__ == "__main__":
    nc = build_nc()
    print("built ok; instructions:", len(nc.inst_map))
